# revision 2
# baseline (speedup 1.0000x reference)
"""Trainium2 Bass kernel for FeatureEmbedding (embedding_lookup).

Reference computation:
  cat_out  = cat_tables[cat_type_ids, cat_value_ids]            # [200000, 128] gather
  cont_out = cont_values[:,None]*cont_w[ct] + cont_b[ct]        # [150000, 128]
  txn_out  = txn_feats @ txn_w + txn_b                          # [150000, 128] GEMM
  return concat([cat_out, cont_out, txn_out], 0)                # [500000, 128]

Strategy (8-way data parallel over the node dimension):
  - cat:  fused idx = type*1000+value (int16, host), SWDGE dma_gather of 512B
          rows DRAM->SBUF, contiguous store. Indices are permuted host-side so
          each SBUF partition holds a contiguous run of output rows, making the
          output DMA per-partition contiguous.
  - cont: host builds M[8, n] with M[t,j]=v_j*(type_j==t) (t<4) and
          M[4+t,j]=(type_j==t); out.T = concat(w,b).T @ M on the PE (K=8).
  - txn:  feats are transposed host-side; out.T = txn_w.T @ featsT on the PE
          (fp32, K=371 in 3 chunks), bias added via DVE per-partition scalar.
  cont/txn outputs are produced transposed ([H, rows]) and un-transposed on
  the host during unsharding.
"""

import numpy as np

import concourse.bass as bass  # noqa: F401  (engine types referenced via nc)
import concourse.mybir as mybir
import concourse.tile as tile
from concourse import bacc
from concourse.bass_utils import run_bass_kernel_spmd

# ---- problem constants (hardcoded per contract) ----
H = 128
VOCAB = 1000
N_CAT, N_CONT, N_TXN = 200_000, 150_000, 150_000
F_TXN = 371
NCORES = 8

# per-core shard sizes
CATC = N_CAT // NCORES          # 25000
CONTC = N_CONT // NCORES        # 18750
TXNC = N_TXN // NCORES          # 18750

# cat gather layout: rows mapped partition-major, processed in chunks
NB_CAT = (CATC + 127) // 128    # 196 blocks of 128 rows -> 25088 padded rows
CAT_PAD = NB_CAT * 128          # 25088
CAT_CHUNK_NB = 28               # blocks per gather chunk
N_CAT_CHUNKS = NB_CAT // CAT_CHUNK_NB   # 7
CAT_CHUNK_IDX = CAT_CHUNK_NB * 128      # 3584 indices per gather

# txn/cont row tiling
RT = 512
NR = (TXNC + RT - 1) // RT      # 37
RPAD = NR * RT                  # 18944
KCH = [(0, 128), (128, 128), (256, F_TXN - 256)]  # K chunks: 128,128,115

_F32 = mybir.dt.float32
_I16 = mybir.dt.int16

_CACHED = {}


def _build_module():
    nc = bacc.Bacc(
        "TRN2",
        target_bir_lowering=False,
        debug=False,
        num_devices=NCORES,
    )

    tables = nc.dram_tensor(
        "tables", [4 * VOCAB, H], _F32, kind="ExternalInput").ap()
    cat_idx = nc.dram_tensor(
        "cat_idx", [128, CAT_PAD // 16], _I16, kind="ExternalInput").ap()
    featsT = nc.dram_tensor(
        "featsT", [F_TXN, RPAD], _F32, kind="ExternalInput").ap()
    w3 = nc.dram_tensor("w3", [384, H], _F32, kind="ExternalInput").ap()
    w8 = nc.dram_tensor("w8", [8, H], _F32, kind="ExternalInput").ap()
    biasT = nc.dram_tensor("biasT", [H, 1], _F32, kind="ExternalInput").ap()
    m8 = nc.dram_tensor("m8", [8, RPAD], _F32, kind="ExternalInput").ap()

    out_cat = nc.dram_tensor(
        "out_cat", [CAT_PAD, H], _F32, kind="ExternalOutput").ap()
    out_cont = nc.dram_tensor(
        "out_cont", [H, RPAD], _F32, kind="ExternalOutput").ap()
    out_txn = nc.dram_tensor(
        "out_txn", [H, RPAD], _F32, kind="ExternalOutput").ap()

    with tile.TileContext(nc) as tc:
        with (
            tc.tile_pool(name="const", bufs=1) as cpool,
            tc.tile_pool(name="catg", bufs=4) as catp,
            tc.tile_pool(name="rhs", bufs=8) as rhsp,
            tc.tile_pool(name="mts", bufs=4) as mpool,
            tc.tile_pool(name="outs", bufs=4) as opool,
            tc.tile_pool(name="ps_txn", bufs=3, space="PSUM") as pp,
            tc.tile_pool(name="ps_cont", bufs=3, space="PSUM") as pp2,
        ):
            idx_t = cpool.tile([128, CAT_PAD // 16], _I16, tag="idx")
            nc.sync.dma_start(idx_t[:], cat_idx)
            w3_t = cpool.tile([128, 3, H], _F32, tag="w3")
            nc.sync.dma_start(w3_t[:], w3.rearrange("(c p) h -> p c h", p=128))
            w8_t = cpool.tile([8, H], _F32, tag="w8")
            nc.sync.dma_start(w8_t[:], w8)
            b_t = cpool.tile([H, 1], _F32, tag="bias")
            nc.sync.dma_start(b_t[:], biasT)

            out_cat3 = out_cat.rearrange("(p g) h -> p g h", p=128)

            def emit_cat_chunk(ci):
                g = catp.tile([128, CAT_CHUNK_NB, H], _F32, tag="catg")
                nc.gpsimd.dma_gather(
                    g[:],
                    tables,
                    idx_t[:, ci * (CAT_CHUNK_IDX // 16):(ci + 1) * (CAT_CHUNK_IDX // 16)],
                    CAT_CHUNK_IDX,
                    CAT_CHUNK_IDX,
                    H,
                    # >64 descriptors per engine overflows the single-packet
                    # limit; 512B descriptors don't benefit from packing anyway
                    single_packet=False,
                )
                nc.scalar.dma_start(
                    out_cat3[:, ci * CAT_CHUNK_NB:(ci + 1) * CAT_CHUNK_NB, :],
                    g[:],
                )

            cat_ci = 0
            for ri in range(NR):
                r0 = ri * RT
                psum = pp.tile([128, RT], _F32, tag="ps")
                for ki, (k0, kp) in enumerate(KCH):
                    rhs_t = rhsp.tile([128, RT], _F32, tag="rhs")
                    nc.sync.dma_start(
                        rhs_t[:kp, :], featsT[k0:k0 + kp, r0:r0 + RT])
                    nc.tensor.matmul(
                        psum[:],
                        w3_t[:kp, ki, :],
                        rhs_t[:kp, :],
                        start=(ki == 0),
                        stop=(ki == len(KCH) - 1),
                    )
                ob = opool.tile([128, RT], _F32, tag="otxn")
                nc.vector.tensor_scalar_add(ob[:], psum[:], b_t[:, 0:1])
                nc.scalar.dma_start(out_txn[:, r0:r0 + RT], ob[:])

                mt = mpool.tile([8, RT], _F32, tag="m")
                nc.sync.dma_start(mt[:], m8[:, r0:r0 + RT])
                ps2 = pp2.tile([128, RT], _F32, tag="ps2")
                nc.tensor.matmul(ps2[:], w8_t[:], mt[:], start=True, stop=True)
                oc = opool.tile([128, RT], _F32, tag="ocont")
                nc.scalar.copy(oc[:], ps2[:])
                nc.scalar.dma_start(out_cont[:, r0:r0 + RT], oc[:])

                # interleave the 7 cat gather chunks across the row loop
                if ri % 5 == 2 and cat_ci < N_CAT_CHUNKS:
                    emit_cat_chunk(cat_ci)
                    cat_ci += 1
            while cat_ci < N_CAT_CHUNKS:
                emit_cat_chunk(cat_ci)
                cat_ci += 1

    nc.compile()
    return nc


def get_module():
    if "nc" not in _CACHED:
        _CACHED["nc"] = _build_module()
    return _CACHED["nc"]


def _prep_in_maps(inputs):
    tables = np.ascontiguousarray(
        np.asarray(inputs["cat_tables"], dtype=np.float32).reshape(4 * VOCAB, H))
    txn_w = np.asarray(inputs["txn_w"], dtype=np.float32)
    w3 = np.zeros((384, H), dtype=np.float32)
    w3[:F_TXN] = txn_w
    w8 = np.ascontiguousarray(np.concatenate(
        [np.asarray(inputs["cont_w"], dtype=np.float32),
         np.asarray(inputs["cont_b"], dtype=np.float32)], axis=0))
    biasT = np.ascontiguousarray(
        np.asarray(inputs["txn_b"], dtype=np.float32).reshape(H, 1))

    cat_type = np.asarray(inputs["cat_type_ids"]).astype(np.int64)
    cat_val = np.asarray(inputs["cat_value_ids"]).astype(np.int64)
    fused_all = (cat_type * VOCAB + cat_val).astype(np.int16)

    cont_type = np.asarray(inputs["cont_type_ids"]).astype(np.int64)
    cont_val = np.asarray(inputs["cont_values"]).astype(np.float32)
    feats = np.asarray(inputs["txn_feats"], dtype=np.float32)

    in_maps = []
    for c in range(NCORES):
        # --- cat indices: permute so partition p holds rows p*NB_CAT..+NB_CAT-1
        f = np.zeros(CAT_PAD, dtype=np.int16)
        f[:CATC] = fused_all[c * CATC:(c + 1) * CATC]
        f2 = f.reshape(128, NB_CAT)                       # [p, g]
        gidx = np.ascontiguousarray(
            f2.reshape(128, N_CAT_CHUNKS, CAT_CHUNK_NB)
              .transpose(1, 2, 0)).reshape(-1)            # gather order
        wrapped = np.ascontiguousarray(gidx.reshape(-1, 16).T)   # [16, n/16]
        cat_idx = np.ascontiguousarray(np.tile(wrapped, (8, 1)))  # [128, n/16]

        # --- txn features, transposed + column padded
        ft = np.zeros((F_TXN, RPAD), dtype=np.float32)
        ft[:, :TXNC] = feats[c * TXNC:(c + 1) * TXNC].T

        # --- cont one-hot matrix M
        t = cont_type[c * CONTC:(c + 1) * CONTC]
        v = cont_val[c * CONTC:(c + 1) * CONTC]
        m = np.zeros((8, RPAD), dtype=np.float32)
        j = np.arange(CONTC)
        m[t, j] = v
        m[4 + t, j] = 1.0

        in_maps.append({
            "tables": tables,
            "cat_idx": cat_idx,
            "featsT": ft,
            "w3": w3,
            "w8": w8,
            "biasT": biasT,
            "m8": m,
        })
    return in_maps


def _assemble(results, out_dtype=np.float32):
    out = np.empty((N_CAT + N_CONT + N_TXN, H), dtype=out_dtype)
    for c, res in enumerate(results):
        out[c * CATC:(c + 1) * CATC] = res["out_cat"][:CATC]
        out[N_CAT + c * CONTC:N_CAT + (c + 1) * CONTC] = \
            res["out_cont"][:, :CONTC].T
        base = N_CAT + N_CONT
        out[base + c * TXNC:base + (c + 1) * TXNC] = \
            res["out_txn"][:, :TXNC].T
    return out


def kernel(_trace=False, **inputs):
    nc = get_module()
    in_maps = _prep_in_maps(inputs)
    res = run_bass_kernel_spmd(
        nc, in_maps, core_ids=list(range(NCORES)), trace=_trace)
    out = _assemble(res.results)
    if _trace:
        _CACHED["last_results"] = res
    return out


# revision 3
# speedup vs baseline: 1.0277x; 1.0277x over previous
"""Trainium2 Bass kernel for FeatureEmbedding (embedding_lookup).

Reference computation:
  cat_out  = cat_tables[cat_type_ids, cat_value_ids]            # [200000, 128] gather
  cont_out = cont_values[:,None]*cont_w[ct] + cont_b[ct]        # [150000, 128]
  txn_out  = txn_feats @ txn_w + txn_b                          # [150000, 128] GEMM
  return concat([cat_out, cont_out, txn_out], 0)                # [500000, 128]

Strategy (8-way data parallel over the node dimension):
  - cat:  fused idx = type*1000+value (int16, host), SWDGE dma_gather of 512B
          rows DRAM->SBUF, contiguous store. Indices are permuted host-side so
          each SBUF partition holds a contiguous run of output rows, making the
          output DMA per-partition contiguous. Gathers are chunked to 896
          indices (56+1 descriptors per DMA engine) so the fast packed
          single_packet SWDGE path stays under the 64-descriptor packet cap.
  - cont: host builds M[8, n] with M[t,j]=v_j*(type_j==t) (t<4) and
          M[4+t,j]=(type_j==t); out.T = concat(w,b).T @ M on the PE (K=8).
  - txn:  feats are transposed host-side; out.T = txn_w.T @ featsT on the PE
          (fp32, K=371 in 3 chunks), bias added via DVE per-partition scalar.
  cont/txn outputs are produced transposed ([H, rows]) and un-transposed on
  the host during unsharding.
"""

import numpy as np

import concourse.bass as bass  # noqa: F401  (engine types referenced via nc)
import concourse.mybir as mybir
import concourse.tile as tile
from concourse import bacc
from concourse.bass_utils import run_bass_kernel_spmd

# ---- problem constants (hardcoded per contract) ----
H = 128
VOCAB = 1000
N_CAT, N_CONT, N_TXN = 200_000, 150_000, 150_000
F_TXN = 371
NCORES = 8

# per-core shard sizes
CATC = N_CAT // NCORES          # 25000
CONTC = N_CONT // NCORES        # 18750
TXNC = N_TXN // NCORES          # 18750

# cat gather layout: rows mapped partition-major, processed in chunks
NB_CAT = (CATC + 127) // 128    # 196 blocks of 128 rows -> 25088 padded rows
CAT_PAD = NB_CAT * 128          # 25088
CAT_CHUNK_NB = 7                # blocks per gather chunk (896 idx: 57 desc/engine <= 64)
N_CAT_CHUNKS = NB_CAT // CAT_CHUNK_NB   # 28
CAT_CHUNK_IDX = CAT_CHUNK_NB * 128      # 896 indices per gather

# txn/cont row tiling: 1024-wide DMA tiles, two 512-wide matmuls each
RT = 512                        # matmul moving-dim tile (one PSUM bank fp32)
BT = 1024                       # DMA tile width (2 matmul tiles)
NB = (TXNC + BT - 1) // BT      # 19 DMA blocks
RPAD = NB * BT                  # 19456
KCH = [(0, 128), (128, 128), (256, F_TXN - 256)]  # K chunks: 128,128,115

_F32 = mybir.dt.float32
_I16 = mybir.dt.int16

_CACHED = {}


def _build_module():
    nc = bacc.Bacc(
        "TRN2",
        target_bir_lowering=False,
        debug=False,
        num_devices=NCORES,
    )

    tables = nc.dram_tensor(
        "tables", [4 * VOCAB, H], _F32, kind="ExternalInput").ap()
    cat_idx = nc.dram_tensor(
        "cat_idx", [128, CAT_PAD // 16], _I16, kind="ExternalInput").ap()
    featsT = nc.dram_tensor(
        "featsT", [F_TXN, RPAD], _F32, kind="ExternalInput").ap()
    w3 = nc.dram_tensor("w3", [384, H], _F32, kind="ExternalInput").ap()
    w8 = nc.dram_tensor("w8", [8, H], _F32, kind="ExternalInput").ap()
    biasT = nc.dram_tensor("biasT", [H, 1], _F32, kind="ExternalInput").ap()
    m8 = nc.dram_tensor("m8", [8, RPAD], _F32, kind="ExternalInput").ap()

    out_cat = nc.dram_tensor(
        "out_cat", [CAT_PAD, H], _F32, kind="ExternalOutput").ap()
    out_cont = nc.dram_tensor(
        "out_cont", [H, RPAD], _F32, kind="ExternalOutput").ap()
    out_txn = nc.dram_tensor(
        "out_txn", [H, RPAD], _F32, kind="ExternalOutput").ap()

    with tile.TileContext(nc) as tc:
        with (
            tc.tile_pool(name="const", bufs=1) as cpool,
            tc.tile_pool(name="catg", bufs=6) as catp,
            tc.tile_pool(name="rhs", bufs=6) as rhsp,
            tc.tile_pool(name="mts", bufs=3) as mpool,
            tc.tile_pool(name="outs", bufs=3) as opool,
            tc.tile_pool(name="ps_txn", bufs=4, space="PSUM") as pp,
            tc.tile_pool(name="ps_cont", bufs=4, space="PSUM") as pp2,
        ):
            idx_t = cpool.tile([128, CAT_PAD // 16], _I16, tag="idx")
            nc.sync.dma_start(idx_t[:], cat_idx)
            w3_t = cpool.tile([128, 3, H], _F32, tag="w3")
            nc.sync.dma_start(w3_t[:], w3.rearrange("(c p) h -> p c h", p=128))
            w8_t = cpool.tile([8, H], _F32, tag="w8")
            nc.sync.dma_start(w8_t[:], w8)
            b_t = cpool.tile([H, 1], _F32, tag="bias")
            nc.sync.dma_start(b_t[:], biasT)

            out_cat3 = out_cat.rearrange("(p g) h -> p g h", p=128)
            icols = CAT_CHUNK_IDX // 16

            def emit_cat_chunk(ci):
                g = catp.tile([128, CAT_CHUNK_NB, H], _F32, tag="catg")
                nc.gpsimd.dma_gather(
                    g[:],
                    tables,
                    idx_t[:, ci * icols:(ci + 1) * icols],
                    CAT_CHUNK_IDX,
                    CAT_CHUNK_IDX,
                    H,
                )
                nc.scalar.dma_start(
                    out_cat3[:, ci * CAT_CHUNK_NB:(ci + 1) * CAT_CHUNK_NB, :],
                    g[:],
                )

            cat_ci = 0
            for bi in range(NB):
                b0 = bi * BT
                # load a 1024-wide slab of featsT (3 K-chunks) and of M
                rhs_t = rhsp.tile([128, 3, BT], _F32, tag="rhs")
                for ki, (k0, kp) in enumerate(KCH):
                    nc.sync.dma_start(
                        rhs_t[:kp, ki, :], featsT[k0:k0 + kp, b0:b0 + BT])
                mt = mpool.tile([8, BT], _F32, tag="m")
                nc.sync.dma_start(mt[:], m8[:, b0:b0 + BT])

                ob = opool.tile([128, BT], _F32, tag="otxn")
                oc = opool.tile([128, BT], _F32, tag="ocont")
                for half in range(2):
                    r0 = half * RT
                    psum = pp.tile([128, RT], _F32, tag="ps")
                    for ki, (k0, kp) in enumerate(KCH):
                        nc.tensor.matmul(
                            psum[:],
                            w3_t[:kp, ki, :],
                            rhs_t[:kp, ki, r0:r0 + RT],
                            start=(ki == 0),
                            stop=(ki == len(KCH) - 1),
                        )
                    nc.vector.tensor_scalar_add(
                        ob[:, r0:r0 + RT], psum[:], b_t[:, 0:1])
                    ps2 = pp2.tile([128, RT], _F32, tag="ps2")
                    nc.tensor.matmul(
                        ps2[:], w8_t[:], mt[:, r0:r0 + RT],
                        start=True, stop=True)
                    nc.scalar.copy(oc[:, r0:r0 + RT], ps2[:])
                nc.scalar.dma_start(out_txn[:, b0:b0 + BT], ob[:])
                nc.scalar.dma_start(out_cont[:, b0:b0 + BT], oc[:])

                # interleave cat gather chunks across the row loop
                while cat_ci * NB < N_CAT_CHUNKS * (bi + 1) \
                        and cat_ci < N_CAT_CHUNKS:
                    emit_cat_chunk(cat_ci)
                    cat_ci += 1
            while cat_ci < N_CAT_CHUNKS:
                emit_cat_chunk(cat_ci)
                cat_ci += 1

    nc.compile()
    return nc


def get_module():
    if "nc" not in _CACHED:
        _CACHED["nc"] = _build_module()
    return _CACHED["nc"]


def _prep_in_maps(inputs):
    tables = np.ascontiguousarray(
        np.asarray(inputs["cat_tables"], dtype=np.float32).reshape(4 * VOCAB, H))
    txn_w = np.asarray(inputs["txn_w"], dtype=np.float32)
    w3 = np.zeros((384, H), dtype=np.float32)
    w3[:F_TXN] = txn_w
    w8 = np.ascontiguousarray(np.concatenate(
        [np.asarray(inputs["cont_w"], dtype=np.float32),
         np.asarray(inputs["cont_b"], dtype=np.float32)], axis=0))
    biasT = np.ascontiguousarray(
        np.asarray(inputs["txn_b"], dtype=np.float32).reshape(H, 1))

    cat_type = np.asarray(inputs["cat_type_ids"]).astype(np.int64)
    cat_val = np.asarray(inputs["cat_value_ids"]).astype(np.int64)
    fused_all = (cat_type * VOCAB + cat_val).astype(np.int16)

    cont_type = np.asarray(inputs["cont_type_ids"]).astype(np.int64)
    cont_val = np.asarray(inputs["cont_values"]).astype(np.float32)
    feats = np.asarray(inputs["txn_feats"], dtype=np.float32)

    in_maps = []
    for c in range(NCORES):
        # --- cat indices: permute so partition p holds rows p*NB_CAT..+NB_CAT-1
        f = np.zeros(CAT_PAD, dtype=np.int16)
        f[:CATC] = fused_all[c * CATC:(c + 1) * CATC]
        f2 = f.reshape(128, NB_CAT)                       # [p, g]
        gidx = np.ascontiguousarray(
            f2.reshape(128, N_CAT_CHUNKS, CAT_CHUNK_NB)
              .transpose(1, 2, 0)).reshape(-1)            # gather order
        wrapped = np.ascontiguousarray(gidx.reshape(-1, 16).T)   # [16, n/16]
        cat_idx = np.ascontiguousarray(np.tile(wrapped, (8, 1)))  # [128, n/16]

        # --- txn features, transposed + column padded
        ft = np.zeros((F_TXN, RPAD), dtype=np.float32)
        ft[:, :TXNC] = feats[c * TXNC:(c + 1) * TXNC].T

        # --- cont one-hot matrix M
        t = cont_type[c * CONTC:(c + 1) * CONTC]
        v = cont_val[c * CONTC:(c + 1) * CONTC]
        m = np.zeros((8, RPAD), dtype=np.float32)
        j = np.arange(CONTC)
        m[t, j] = v
        m[4 + t, j] = 1.0

        in_maps.append({
            "tables": tables,
            "cat_idx": cat_idx,
            "featsT": ft,
            "w3": w3,
            "w8": w8,
            "biasT": biasT,
            "m8": m,
        })
    return in_maps


def _assemble(results, out_dtype=np.float32):
    out = np.empty((N_CAT + N_CONT + N_TXN, H), dtype=out_dtype)
    for c, res in enumerate(results):
        out[c * CATC:(c + 1) * CATC] = res["out_cat"][:CATC]
        out[N_CAT + c * CONTC:N_CAT + (c + 1) * CONTC] = \
            res["out_cont"][:, :CONTC].T
        base = N_CAT + N_CONT
        out[base + c * TXNC:base + (c + 1) * TXNC] = \
            res["out_txn"][:, :TXNC].T
    return out


def kernel(_trace=False, **inputs):
    nc = get_module()
    in_maps = _prep_in_maps(inputs)
    res = run_bass_kernel_spmd(
        nc, in_maps, core_ids=list(range(NCORES)), trace=_trace)
    out = _assemble(res.results)
    if _trace:
        _CACHED["last_results"] = res
    return out


# revision 5
# speedup vs baseline: 1.1015x; 1.0719x over previous
"""Trainium2 Bass kernel for FeatureEmbedding (embedding_lookup).

Reference computation:
  cat_out  = cat_tables[cat_type_ids, cat_value_ids]            # [200000, 128] gather
  cont_out = cont_values[:,None]*cont_w[ct] + cont_b[ct]        # [150000, 128]
  txn_out  = txn_feats @ txn_w + txn_b                          # [150000, 128] GEMM
  return concat([cat_out, cont_out, txn_out], 0)                # [500000, 128]

Strategy (8-way data parallel over the node dimension):
  - cat:  fused idx = type*1000+value (int16, host), SWDGE dma_gather of 512B
          rows DRAM->SBUF spread over all 4 SWDGE queues (the Q7 descriptor
          generation is the gather bottleneck; queues run concurrently),
          then a contiguous store. Indices are permuted host-side so each
          SBUF partition holds a contiguous run of output rows. 896-idx
          chunks keep the packed single-packet path under the 64-descriptor
          per-engine packet cap.
  - txn:  feats are transposed host-side and split into bf16 hi/lo pairs
          (x = hi + lo). out.T = txn_w.T @ featsT on the PE with 3 bf16
          accumulating terms (hi@hi + lo@hi + hi@lo) per K-chunk — fp32-grade
          accuracy at bf16 matmul throughput (fp32 PE matmul is 4x slower and
          was the critical path). Bias added via DVE per-partition scalar.
  - cont: host builds M[8, n] with M[t,j]=v_j*(type_j==t) (t<4) and
          M[4+t,j]=(type_j==t); out.T = concat(w,b).T @ M on the PE (K=8),
          same bf16 hi/lo scheme.
  cont/txn outputs are produced transposed ([H, rows]) and un-transposed on
  the host during unsharding.
"""

import ml_dtypes
import numpy as np

import concourse.bass as bass  # noqa: F401  (engine types referenced via nc)
import concourse.mybir as mybir
import concourse.tile as tile
from concourse import bacc
from concourse.bass_utils import run_bass_kernel_spmd

BF16 = ml_dtypes.bfloat16

# ---- problem constants (hardcoded per contract) ----
H = 128
VOCAB = 1000
N_CAT, N_CONT, N_TXN = 200_000, 150_000, 150_000
F_TXN = 371
NCORES = 8

# per-core shard sizes
CATC = N_CAT // NCORES          # 25000
CONTC = N_CONT // NCORES        # 18750
TXNC = N_TXN // NCORES          # 18750

# cat gather layout: rows mapped partition-major, processed in chunks
NB_CAT = (CATC + 127) // 128    # 196 blocks of 128 rows -> 25088 padded rows
CAT_PAD = NB_CAT * 128          # 25088
CAT_CHUNK_NB = 7                # blocks per gather chunk (896 idx: 57 desc/engine <= 64)
N_CAT_CHUNKS = NB_CAT // CAT_CHUNK_NB   # 28
CAT_CHUNK_IDX = CAT_CHUNK_NB * 128      # 896 indices per gather

# txn/cont row tiling: 1024-wide DMA tiles, two 512-wide matmuls each
RT = 512                        # matmul moving-dim tile (one PSUM bank fp32)
BT = 1024                       # DMA tile width (2 matmul tiles)
NB = (TXNC + BT - 1) // BT      # 19 DMA blocks
RPAD = NB * BT                  # 19456
KCH = [(0, 128), (128, 128), (256, F_TXN - 256)]  # K chunks: 128,128,115

_F32 = mybir.dt.float32
_BF16 = mybir.dt.bfloat16
_I16 = mybir.dt.int16

_CACHED = {}


def _build_module():
    nc = bacc.Bacc(
        "TRN2",
        target_bir_lowering=False,
        debug=False,
        num_devices=NCORES,
        num_swdge_queues=4,
    )

    tables = nc.dram_tensor(
        "tables", [4 * VOCAB, H], _F32, kind="ExternalInput").ap()
    cat_idx = nc.dram_tensor(
        "cat_idx", [128, CAT_PAD // 16], _I16, kind="ExternalInput").ap()
    # hi/lo bf16 pairs, packed on a leading axis (one DMA loads both)
    feats2 = nc.dram_tensor(
        "feats2", [2, F_TXN, RPAD], _BF16, kind="ExternalInput").ap()
    w32 = nc.dram_tensor("w32", [2, 384, H], _BF16, kind="ExternalInput").ap()
    w82 = nc.dram_tensor("w82", [2, 8, H], _BF16, kind="ExternalInput").ap()
    biasT = nc.dram_tensor("biasT", [H, 1], _F32, kind="ExternalInput").ap()
    m82 = nc.dram_tensor("m82", [2, 8, RPAD], _BF16, kind="ExternalInput").ap()

    out_cat = nc.dram_tensor(
        "out_cat", [CAT_PAD, H], _F32, kind="ExternalOutput").ap()
    out_cont = nc.dram_tensor(
        "out_cont", [H, RPAD], _F32, kind="ExternalOutput").ap()
    out_txn = nc.dram_tensor(
        "out_txn", [H, RPAD], _F32, kind="ExternalOutput").ap()

    with tile.TileContext(nc) as tc:
        with (
            tc.tile_pool(name="const", bufs=1) as cpool,
            tc.tile_pool(name="catg", bufs=8) as catp,
            tc.tile_pool(name="rhs", bufs=6) as rhsp,
            tc.tile_pool(name="mts", bufs=3) as mpool,
            tc.tile_pool(name="outs", bufs=3) as opool,
            tc.tile_pool(name="ps_txn", bufs=4, space="PSUM") as pp,
            tc.tile_pool(name="ps_cont", bufs=4, space="PSUM") as pp2,
        ):
            idx_t = cpool.tile([128, CAT_PAD // 16], _I16, tag="idx")
            nc.sync.dma_start(idx_t[:], cat_idx)
            # w3_t[p, ki, h, :] = w32[h, ki*128+p, :]
            w3_t = cpool.tile([128, 3, 2, H], _BF16, tag="w3")
            for h in range(2):
                nc.sync.dma_start(
                    w3_t[:, :, h, :],
                    w32[h].rearrange("(c p) x -> p c x", p=128))
            w8_t = cpool.tile([8, 2, H], _BF16, tag="w8")
            for h in range(2):
                nc.sync.dma_start(w8_t[:, h, :], w82[h])
            b_t = cpool.tile([H, 1], _F32, tag="bias")
            nc.sync.dma_start(b_t[:], biasT)

            out_cat3 = out_cat.rearrange("(p g) h -> p g h", p=128)
            icols = CAT_CHUNK_IDX // 16

            def emit_cat_chunk(ci):
                g = catp.tile([128, CAT_CHUNK_NB, H], _F32, tag="catg")
                nc.gpsimd.dma_gather(
                    g[:],
                    tables,
                    idx_t[:, ci * icols:(ci + 1) * icols],
                    CAT_CHUNK_IDX,
                    CAT_CHUNK_IDX,
                    H,
                    queue_num=ci % 4,
                )
                nc.scalar.dma_start(
                    out_cat3[:, ci * CAT_CHUNK_NB:(ci + 1) * CAT_CHUNK_NB, :],
                    g[:],
                )

            # (weight-half, x-half) terms: hi@hi + lo@hi + hi@lo
            TERMS = [(0, 0), (1, 0), (0, 1)]

            cat_ci = 0
            for bi in range(NB):
                b0 = bi * BT
                # load a 1024-wide hi+lo slab of featsT (3 K-chunks) and of M
                rhs_t = rhsp.tile([128, 3, 2, BT], _BF16, tag="rhs")
                for ki, (k0, kp) in enumerate(KCH):
                    nc.sync.dma_start(
                        rhs_t[:kp, ki, :, :], feats2[:, k0:k0 + kp, b0:b0 + BT]
                        .rearrange("h p x -> p h x"))
                mt = mpool.tile([8, 2, BT], _BF16, tag="m")
                nc.sync.dma_start(
                    mt[:], m82[:, :, b0:b0 + BT].rearrange("h c x -> c h x"))

                ob = opool.tile([128, BT], _F32, tag="otxn")
                oc = opool.tile([128, BT], _F32, tag="ocont")
                for half in range(2):
                    r0 = half * RT
                    psum = pp.tile([128, RT], _F32, tag="ps")
                    n_mm = len(TERMS) * len(KCH)
                    i_mm = 0
                    for wh, xh in TERMS:
                        for ki, (k0, kp) in enumerate(KCH):
                            nc.tensor.matmul(
                                psum[:],
                                w3_t[:kp, ki, wh, :],
                                rhs_t[:kp, ki, xh, r0:r0 + RT],
                                start=(i_mm == 0),
                                stop=(i_mm == n_mm - 1),
                            )
                            i_mm += 1
                    nc.vector.tensor_scalar_add(
                        ob[:, r0:r0 + RT], psum[:], b_t[:, 0:1])
                    ps2 = pp2.tile([128, RT], _F32, tag="ps2")
                    for i_mm, (wh, xh) in enumerate(TERMS):
                        nc.tensor.matmul(
                            ps2[:], w8_t[:, wh, :], mt[:, xh, r0:r0 + RT],
                            start=(i_mm == 0), stop=(i_mm == len(TERMS) - 1))
                    nc.scalar.copy(oc[:, r0:r0 + RT], ps2[:])
                nc.scalar.dma_start(out_txn[:, b0:b0 + BT], ob[:])
                nc.scalar.dma_start(out_cont[:, b0:b0 + BT], oc[:])

                # interleave cat gather chunks across the row loop
                while cat_ci * NB < N_CAT_CHUNKS * (bi + 1) \
                        and cat_ci < N_CAT_CHUNKS:
                    emit_cat_chunk(cat_ci)
                    cat_ci += 1
            while cat_ci < N_CAT_CHUNKS:
                emit_cat_chunk(cat_ci)
                cat_ci += 1

    nc.compile()
    return nc


def get_module():
    if "nc" not in _CACHED:
        _CACHED["nc"] = _build_module()
    return _CACHED["nc"]


def _hilo(x):
    """Split fp32 array into (hi, lo) bf16 so hi+lo ~= x to ~2^-17."""
    hi = x.astype(BF16)
    lo = (x - hi.astype(np.float32)).astype(BF16)
    return hi, lo


def _prep_in_maps(inputs):
    tables = np.ascontiguousarray(
        np.asarray(inputs["cat_tables"], dtype=np.float32).reshape(4 * VOCAB, H))
    txn_w = np.asarray(inputs["txn_w"], dtype=np.float32)
    w3f = np.zeros((384, H), dtype=np.float32)
    w3f[:F_TXN] = txn_w
    w32 = np.stack(_hilo(w3f), axis=0)                       # [2, 384, H] bf16
    w8f = np.concatenate(
        [np.asarray(inputs["cont_w"], dtype=np.float32),
         np.asarray(inputs["cont_b"], dtype=np.float32)], axis=0)
    w82 = np.stack(_hilo(w8f), axis=0)                       # [2, 8, H] bf16
    biasT = np.ascontiguousarray(
        np.asarray(inputs["txn_b"], dtype=np.float32).reshape(H, 1))

    cat_type = np.asarray(inputs["cat_type_ids"]).astype(np.int64)
    cat_val = np.asarray(inputs["cat_value_ids"]).astype(np.int64)
    fused_all = (cat_type * VOCAB + cat_val).astype(np.int16)

    cont_type = np.asarray(inputs["cont_type_ids"]).astype(np.int64)
    cont_val = np.asarray(inputs["cont_values"]).astype(np.float32)
    feats = np.asarray(inputs["txn_feats"], dtype=np.float32)

    in_maps = []
    for c in range(NCORES):
        # --- cat indices: permute so partition p holds rows p*NB_CAT..+NB_CAT-1
        f = np.zeros(CAT_PAD, dtype=np.int16)
        f[:CATC] = fused_all[c * CATC:(c + 1) * CATC]
        f2 = f.reshape(128, NB_CAT)                       # [p, g]
        gidx = np.ascontiguousarray(
            f2.reshape(128, N_CAT_CHUNKS, CAT_CHUNK_NB)
              .transpose(1, 2, 0)).reshape(-1)            # gather order
        wrapped = np.ascontiguousarray(gidx.reshape(-1, 16).T)   # [16, n/16]
        cat_idx = np.ascontiguousarray(np.tile(wrapped, (8, 1)))  # [128, n/16]

        # --- txn features, transposed + column padded, bf16 hi/lo
        ft = np.zeros((F_TXN, RPAD), dtype=np.float32)
        ft[:, :TXNC] = feats[c * TXNC:(c + 1) * TXNC].T
        feats2 = np.stack(_hilo(ft), axis=0)              # [2, F, RPAD] bf16

        # --- cont one-hot matrix M, bf16 hi/lo
        t = cont_type[c * CONTC:(c + 1) * CONTC]
        v = cont_val[c * CONTC:(c + 1) * CONTC]
        m = np.zeros((8, RPAD), dtype=np.float32)
        j = np.arange(CONTC)
        m[t, j] = v
        m[4 + t, j] = 1.0
        m82 = np.stack(_hilo(m), axis=0)                  # [2, 8, RPAD] bf16

        in_maps.append({
            "tables": tables,
            "cat_idx": cat_idx,
            "feats2": feats2,
            "w32": w32,
            "w82": w82,
            "biasT": biasT,
            "m82": m82,
        })
    return in_maps


def _assemble(results, out_dtype=np.float32):
    out = np.empty((N_CAT + N_CONT + N_TXN, H), dtype=out_dtype)
    for c, res in enumerate(results):
        out[c * CATC:(c + 1) * CATC] = res["out_cat"][:CATC]
        out[N_CAT + c * CONTC:N_CAT + (c + 1) * CONTC] = \
            res["out_cont"][:, :CONTC].T
        base = N_CAT + N_CONT
        out[base + c * TXNC:base + (c + 1) * TXNC] = \
            res["out_txn"][:, :TXNC].T
    return out


def kernel(_trace=False, **inputs):
    nc = get_module()
    in_maps = _prep_in_maps(inputs)
    res = run_bass_kernel_spmd(
        nc, in_maps, core_ids=list(range(NCORES)), trace=_trace)
    out = _assemble(res.results)
    if _trace:
        _CACHED["last_results"] = res
    return out
